# revision 9
# baseline (speedup 1.0000x reference)
"""Trainium2 Bass kernel for nn_BLoraLinear (batched multi-adapter LoRA linear).

Math:  out = x @ W.T + b + sum_s sum_m mask_s(t) * (x @ A[m,s]) @ B[m,s]

Reformulation (exact): with per-(module,segment) adapter columns packed
into Ahat [D_IN, r_hat] / Bhat [r_hat, D_OUT] and a per-token segment
mask MT [r_hat, T],
    out = x @ W.T + b + ((x @ Ahat) * MT.T) @ Bhat

Sharding: data-parallel over tokens, 1024 tokens per core, zero
collectives.  Since the host knows cu_seqlen values, each core packs
only the adapters of segments overlapping its token range (slots).  Up
to 4 active segments -> r_hat=128; rare draws with more fall back to a
precompiled r_hat=256 variant (always exact).

Precision: the first 2*NP of the 32 k-chunks of the base matmul run as
fp8-e4m3 DoubleRow pair-matmuls (2 MACs/cell/cycle), the rest bf16; the
LoRA down-projection (phase A) runs fully in DoubleRow fp8 (its output
feeds a ~10%-magnitude correction, so fp8 noise there is negligible).
W/Ahat are pre-scaled by 64 into e4m3 range; PSUM accumulates 64*out
and eviction applies *1/64 + bias in one fused DVE op.  Measured
end-to-end rel-err ~1.7e-2 vs the f32 reference (gate 2e-2); bf16-only
is ~2.4e-3.
"""

import numpy as np
import ml_dtypes

# Problem shape (hardcoded per spec nn_BLoraLinear_46471546143180).
T, D_IN, D_OUT, R, M, S = 8192, 4096, 4096, 16, 2, 8
N_CORES = 8
T_C = T // N_CORES
MR = M * R                    # adapter columns per segment (32)

NP = 3                        # k-chunk pairs of the base matmul in fp8 DR
WS = 64.0                     # W / Ahat scale into e4m3 range

BF16 = ml_dtypes.bfloat16
F8 = ml_dtypes.float8_e4m3fn


def _build(t_c, d_in, d_out, r_hat, n_pairs):
    """Per-core Bass/Tile program (same NEFF on all cores).

    DRAM layouts are host-prearranged so every DMA is contiguous per
    partition:
      x8   [128, KX, t_c]       e4m3(x)   x8[p,a,t] = x[tok0+t, a*128+p]
      xb   [128, KB, t_c]       bf16 x, chunks NP2..KX-1 only
      w8   [NB, 128, NP2, 512]  e4m3(64*W.T), chunks 0..NP2-1
      wb   [NB, 128, KB, 512]   bf16(64*W.T), chunks NP2..KX-1
      ah8  [128, KX, r_hat]     e4m3(64*Ahat)
      bh   [128, RC, NB, 512]   bf16 Bhat (unscaled)
      mt   [128, RC, t_c]       bf16 segment mask
      brep [128, d_out]         bf16 bias (unscaled) replicated
      out  [t_c, d_out]         f32
    """
    import concourse.bacc as bacc
    import concourse.mybir as mybir
    from concourse.tile import TileContext

    dt = mybir.dt
    DR = mybir.MatmulPerfMode.DoubleRow
    KX = d_in // 128
    NP2 = 2 * n_pairs
    KB = KX - NP2
    RC = r_hat // 128
    NB = d_out // 512
    MB = t_c // 128
    TB = t_c // 512
    INV = 1.0 / WS

    nc = bacc.Bacc("TRN2", target_bir_lowering=False)

    x8 = nc.dram_tensor("x8", [128, KX, t_c], dt.float8e4, kind="ExternalInput")
    xb = nc.dram_tensor("xb", [128, KB, t_c], dt.bfloat16, kind="ExternalInput")
    w8 = nc.dram_tensor("w8", [NB, 128, NP2, 512], dt.float8e4,
                        kind="ExternalInput")
    wb = nc.dram_tensor("wb", [NB, 128, KB, 512], dt.bfloat16,
                        kind="ExternalInput")
    ah8 = nc.dram_tensor("ah8", [128, KX, r_hat], dt.float8e4,
                         kind="ExternalInput")
    bh = nc.dram_tensor("bh", [128, RC, NB, 512], dt.bfloat16,
                        kind="ExternalInput")
    mt = nc.dram_tensor("mt", [128, RC, t_c], dt.bfloat16, kind="ExternalInput")
    brep = nc.dram_tensor("brep", [128, d_out], dt.bfloat16,
                          kind="ExternalInput")
    out = nc.dram_tensor("out", [t_c, d_out], dt.float32, kind="ExternalOutput")

    with TileContext(nc) as tc:
        with tc.tile_pool(name="resident", bufs=1) as res_pool, \
             tc.tile_pool(name="wpool", bufs=2) as w_pool, \
             tc.tile_pool(name="ps", bufs=8, space="PSUM") as ps_pool, \
             tc.tile_pool(name="opool", bufs=4) as o_pool:
            x8_sb = res_pool.tile([128, KX, t_c], dt.float8e4, name="x8_sb")
            xb_sb = res_pool.tile([128, KB, t_c], dt.bfloat16, name="xb_sb")
            ah8_sb = res_pool.tile([128, KX, r_hat], dt.float8e4, name="ah8_sb")
            bh_sb = res_pool.tile([128, RC, NB, 512], dt.bfloat16, name="bh_sb")
            mt_sb = res_pool.tile([128, RC, t_c], dt.bfloat16, name="mt_sb")
            ut_sb = res_pool.tile([128, RC, t_c], dt.bfloat16, name="ut_sb")
            brep_sb = res_pool.tile([128, d_out], dt.bfloat16, name="brep_sb")

            def wtiles():
                t8 = w_pool.tile([128, NP2, 512], dt.float8e4, name="w8n",
                                 tag="w8n")
                tbf = w_pool.tile([128, KB, 512], dt.bfloat16, name="wbn",
                                  tag="wbn")
                return t8, tbf

            wn_tiles = {}

            def load_wn(n):
                t8, tbf = wtiles()
                nc.sync.dma_start(out=t8[:], in_=w8[n])
                nc.sync.dma_start(out=tbf[:], in_=wb[n])
                wn_tiles[n] = (t8, tbf)

            # PE warm-up: ~13 no-dep matmuls on a scratch tile fill the
            # initial DMA wait and ramp the HAM clock gate to 8/8 before
            # real work arrives.  Results land in a discarded PSUM bank.
            warm_sb = res_pool.tile([128, 640], dt.bfloat16, name="warm_sb")
            nc.vector.memset(warm_sb[:], 0.0)
            ps_w = ps_pool.tile([128, 512], dt.float32, name="ps_w", tag="ps")
            for i in range(13):
                nc.tensor.matmul(ps_w[:], warm_sb[:, 0:128], warm_sb[:, 128:640],
                                 start=(i == 0), stop=(i == 12))

            # Startup is HBM-bandwidth-bound; issue order tracks the PE's
            # consumption order, and transfers are batched to ~1 MiB (small
            # DMAs are descriptor-dominated: 128 KB ~ 180 GB/s vs 1 MB ~ 340).
            step = 4
            h0 = 512
            t8_0, tb_0 = wtiles()
            nc.sync.dma_start(out=ah8_sb[:], in_=ah8[:])
            nc.sync.dma_start(out=x8_sb[:, 0:16, 0:h0], in_=x8[:, 0:16, 0:h0])
            nc.sync.dma_start(out=t8_0[:], in_=w8[0])
            wn_tiles[0] = (t8_0, tb_0)
            nc.sync.dma_start(out=x8_sb[:, 16:KX, 0:h0], in_=x8[:, 16:KX, 0:h0])
            nc.sync.dma_start(out=mt_sb[:], in_=mt[:])
            for g0, g1 in [(0, 4), (4, 10), (10, 18), (18, KB)]:
                nc.sync.dma_start(out=tb_0[:, g0:g1, :], in_=wb[0, :, g0:g1, :])
                nc.sync.dma_start(out=xb_sb[:, g0:g1, 0:h0],
                                  in_=xb[:, g0:g1, 0:h0])
            nc.sync.dma_start(out=x8_sb[:, 0:16, h0:], in_=x8[:, 0:16, h0:])
            nc.sync.dma_start(out=x8_sb[:, 16:KX, h0:], in_=x8[:, 16:KX, h0:])
            nc.sync.dma_start(out=bh_sb[:], in_=bh[:])
            nc.sync.dma_start(out=brep_sb[:], in_=brep[:])
            for g0 in range(0, KB, 13):
                g1 = min(g0 + 13, KB)
                nc.sync.dma_start(out=xb_sb[:, g0:g1, h0:],
                                  in_=xb[:, g0:g1, h0:])

            # Phase A (one tb block): uT[j,t] = 64*mask[j,t]*sum_k Ahat[k,j]x[t,k]
            def phase_a(tb):
                for rc in range(RC):
                    ps_u = ps_pool.tile([128, 512], dt.float32, name="ps_u",
                                        tag="ps")
                    for p in range(KX // 2):
                        nc.tensor.matmul(
                            ps_u[:],
                            ah8_sb[:, 2 * p:2 * p + 2, rc * 128:(rc + 1) * 128],
                            x8_sb[:, 2 * p:2 * p + 2, tb * 512:(tb + 1) * 512],
                            start=(p == 0), stop=(p == KX // 2 - 1),
                            perf_mode=DR,
                        )
                    nc.vector.tensor_mul(
                        out=ut_sb[:, rc, tb * 512:(tb + 1) * 512],
                        in0=ps_u[:],
                        in1=mt_sb[:, rc, tb * 512:(tb + 1) * 512],
                    )

            def lora_mms(n, m, ps_o):
                for r in range(RC):
                    nc.tensor.matmul(
                        ps_o[:],
                        ut_sb[:, r, m * 128:(m + 1) * 128],
                        bh_sb[:, r, n, :],
                        start=False, stop=(r == RC - 1),
                    )

            def evict(n, m, ps_o):
                o_sb = o_pool.tile([128, 512], dt.float32, name="o_sb")
                nc.vector.scalar_tensor_tensor(
                    out=o_sb[:], in0=ps_o[:], scalar=INV,
                    in1=brep_sb[:, n * 512:(n + 1) * 512],
                    op0=mybir.AluOpType.mult, op1=mybir.AluOpType.add,
                )
                nc.sync.dma_start(
                    out=out[m * 128:(m + 1) * 128, n * 512:(n + 1) * 512],
                    in_=o_sb[:],
                )

            # Phase B tile: psum = 64*(x@W.T + u@Bhat)[m-tile, n-tile]
            def phase_b_tile(n, m, w8n, wbn):
                ps_o = ps_pool.tile([128, 512], dt.float32, name="ps_o",
                                    tag="ps")
                for p in range(n_pairs):
                    nc.tensor.matmul(
                        ps_o[:],
                        x8_sb[:, 2 * p:2 * p + 2, m * 128:(m + 1) * 128],
                        w8n[:, 2 * p:2 * p + 2, :],
                        start=(p == 0), stop=False, perf_mode=DR,
                    )
                for kb in range(KB):
                    nc.tensor.matmul(
                        ps_o[:],
                        xb_sb[:, kb, m * 128:(m + 1) * 128],
                        wbn[:, kb, :],
                        start=(n_pairs == 0 and kb == 0), stop=False,
                    )
                lora_mms(n, m, ps_o)
                evict(n, m, ps_o)

            # Prefix: phase A tb0 and phase-B (n=0, m<half) DR parts ride the
            # ah8/x8-h0 windows; then the m0..3 bf16 k-loop tracks the
            # wbn0/xb chunk stream; phase A tb1 and the lora/evicts follow.
            mb_half = MB // TB
            ps_a = [ps_pool.tile([128, 512], dt.float32, name="ps_u", tag="ps")
                    for _ in range(RC)]
            ps_b = [ps_pool.tile([128, 512], dt.float32, name="ps_o", tag="ps")
                    for _ in range(mb_half)]
            def phase_a_pairs(p0, p1):
                for p in range(p0, p1):
                    for rc in range(RC):
                        nc.tensor.matmul(
                            ps_a[rc][:],
                            ah8_sb[:, 2 * p:2 * p + 2, rc * 128:(rc + 1) * 128],
                            x8_sb[:, 2 * p:2 * p + 2, 0:512],
                            start=(p == 0), stop=(p == KX // 2 - 1),
                            perf_mode=DR,
                        )

            phase_a_pairs(0, 8)
            for m in range(mb_half):
                for p in range(n_pairs):
                    nc.tensor.matmul(
                        ps_b[m][:],
                        x8_sb[:, 2 * p:2 * p + 2, m * 128:(m + 1) * 128],
                        t8_0[:, 2 * p:2 * p + 2, :],
                        start=(p == 0), stop=False, perf_mode=DR,
                    )
            for kb in range(4):
                for m in range(mb_half):
                    nc.tensor.matmul(
                        ps_b[m][:],
                        xb_sb[:, kb, m * 128:(m + 1) * 128],
                        tb_0[:, kb, :],
                        start=(n_pairs == 0 and kb == 0), stop=False,
                    )
            phase_a_pairs(8, KX // 2)
            for rc in range(RC):
                nc.vector.tensor_mul(
                    out=ut_sb[:, rc, 0:512], in0=ps_a[rc][:],
                    in1=mt_sb[:, rc, 0:512])
            for kb in range(4, KB):
                for m in range(mb_half):
                    nc.tensor.matmul(
                        ps_b[m][:],
                        xb_sb[:, kb, m * 128:(m + 1) * 128],
                        tb_0[:, kb, :],
                        start=(n_pairs == 0 and kb == 0), stop=False,
                    )
            for tb in range(1, TB):
                phase_a(tb)
            for m in range(mb_half):
                lora_mms(0, m, ps_b[m])
                evict(0, m, ps_b[m])

            # Steady state: remaining tiles.
            load_wn(1)
            for m in range(mb_half, MB):
                phase_b_tile(0, m, t8_0, tb_0)
            for n in range(1, NB):
                if n + 1 < NB:
                    load_wn(n + 1)
                w8n, wbn = wn_tiles.pop(n)
                for m in range(MB):
                    phase_b_tile(n, m, w8n, wbn)

    nc.compile()
    nc.finalize()
    return nc


def _core_slots(cu, t_c, n_cores, n_slots):
    """Per-core list of segments overlapping the core's token range,
    padded with -1 to n_slots.  Returns None if any core needs more."""
    out = []
    for c in range(n_cores):
        lo, hi = c * t_c, (c + 1) * t_c
        slots = [s for s in range(S) if cu[s] < hi and cu[s + 1] > lo
                 and cu[s + 1] > cu[s]]
        if len(slots) > n_slots:
            return None
        out.append(slots + [-1] * (n_slots - len(slots)))
    return out


def _prep_in_maps(x, W, b, lora_A, lora_B, cu_seqlen):
    x = np.asarray(x, dtype=np.float32)
    W = np.asarray(W, dtype=np.float32)
    b = np.asarray(b, dtype=np.float32)
    lora_A = np.asarray(lora_A, dtype=np.float32)
    lora_B = np.asarray(lora_B, dtype=np.float32)
    cu = np.asarray(cu_seqlen).astype(np.int64)

    # full Ahat[k, j], Bhat[j, d], j = (s*M + m)*R + r
    Ahat = np.transpose(lora_A, (2, 1, 0, 3)).reshape(D_IN, S * MR)
    Bhat = np.transpose(lora_B, (1, 0, 2, 3)).reshape(S * MR, D_OUT).astype(BF16)

    r_hat = 128
    slots = _core_slots(cu, T_C, N_CORES, r_hat // MR)
    if slots is None:
        r_hat = S * MR                                   # 256 fallback
        slots = [list(range(S)) for _ in range(N_CORES)]

    KX = D_IN // 128
    NP2 = 2 * NP
    KB = KX - NP2
    RC = r_hat // 128
    NB = D_OUT // 512

    WT = np.ascontiguousarray(W.T) * np.float32(WS)      # [D_IN, D_OUT] scaled
    w8_host = np.ascontiguousarray(
        WT[:NP2 * 128].astype(F8).reshape(NP2, 128, NB, 512)
        .transpose(2, 1, 0, 3))
    wb_host = np.ascontiguousarray(
        WT[NP2 * 128:].astype(BF16).reshape(KB, 128, NB, 512)
        .transpose(2, 1, 0, 3))
    brep_host = np.ascontiguousarray(
        np.broadcast_to(b.astype(BF16), (128, D_OUT)))

    xT = x.T                                             # [D_IN, T] view
    in_maps = []
    for c in range(N_CORES):
        sl = slice(c * T_C, (c + 1) * T_C)
        xs = xT[:, sl]
        x8_host = np.ascontiguousarray(
            xs.astype(F8).reshape(KX, 128, T_C).transpose(1, 0, 2))
        xb_host = np.ascontiguousarray(
            xs[NP2 * 128:].astype(BF16).reshape(KB, 128, T_C)
            .transpose(1, 0, 2))

        Ah_c = np.zeros((D_IN, r_hat), dtype=np.float32)
        Bh_c = np.zeros((r_hat, D_OUT), dtype=BF16)
        MT_c = np.zeros((r_hat, T_C), dtype=BF16)
        for a, s in enumerate(slots[c]):
            if s < 0:
                continue
            Ah_c[:, a * MR:(a + 1) * MR] = Ahat[:, s * MR:(s + 1) * MR]
            Bh_c[a * MR:(a + 1) * MR, :] = Bhat[s * MR:(s + 1) * MR, :]
            lo = max(int(cu[s]) - c * T_C, 0)
            hi = min(int(cu[s + 1]) - c * T_C, T_C)
            if hi > lo:
                MT_c[a * MR:(a + 1) * MR, lo:hi] = 1.0

        ah8_host = np.ascontiguousarray(
            (Ah_c * np.float32(WS)).astype(F8).reshape(KX, 128, r_hat)
            .transpose(1, 0, 2))
        bh_host = np.ascontiguousarray(
            Bh_c.reshape(RC, 128, NB, 512).transpose(1, 0, 2, 3))
        mt_host = np.ascontiguousarray(
            MT_c.reshape(RC, 128, T_C).transpose(1, 0, 2))
        in_maps.append({
            "x8": x8_host, "xb": xb_host, "w8": w8_host, "wb": wb_host,
            "ah8": ah8_host, "bh": bh_host, "mt": mt_host, "brep": brep_host,
        })
    return in_maps, r_hat


_NC_CACHE = {}


def _get_nc(r_hat):
    key = (T_C, D_IN, D_OUT, r_hat, NP)
    if key not in _NC_CACHE:
        _NC_CACHE[key] = _build(T_C, D_IN, D_OUT, r_hat, NP)
    return _NC_CACHE[key]


def _ensure_axon_hooks():
    """concourse's trace path imports antenv.axon_hooks, which this image
    lacks.  Provide the tiny get/set registry and wire it to the PJRT
    .so's NTFF entry points when available; degrade to a None hook."""
    import sys
    import types
    if "antenv.axon_hooks" in sys.modules:
        return
    try:
        mod = types.ModuleType("antenv.axon_hooks")
        mod._hook = None
        mod.set_axon_ntff_profile_hook = lambda h: setattr(mod, "_hook", h)
        mod.get_axon_ntff_profile_hook = lambda: mod._hook
        sys.modules["antenv.axon_hooks"] = mod
        import antenv
        antenv.axon_hooks = mod
        try:
            from trn_agent_boot.trn_boot import _ntff_profile_via_ctypes
            mod._hook = _ntff_profile_via_ctypes("/opt/axon/libaxon_pjrt.so")
        except Exception:
            pass
    except Exception:
        pass


def run(inputs, trace=False):
    """Run the SPMD kernel on 8 cores; returns (full_output, results_obj)."""
    _ensure_axon_hooks()
    from concourse.bass_utils import run_bass_kernel_spmd

    in_maps, r_hat = _prep_in_maps(**inputs)
    nc = _get_nc(r_hat)
    res = run_bass_kernel_spmd(
        nc, in_maps, core_ids=list(range(N_CORES)), trace=trace)
    out = np.concatenate([r["out"] for r in res.results], axis=0)
    return out, res


def kernel(x, W, b, lora_A, lora_B, cu_seqlen):
    out, _ = run(dict(x=x, W=W, b=b, lora_A=lora_A, lora_B=lora_B,
                      cu_seqlen=cu_seqlen))
    return out


# revision 11
# speedup vs baseline: 1.0153x; 1.0153x over previous
"""Trainium2 Bass kernel for nn_BLoraLinear (batched multi-adapter LoRA linear).

Math:  out = x @ W.T + b + sum_s sum_m mask_s(t) * (x @ A[m,s]) @ B[m,s]

Reformulation (exact): with per-(module,segment) adapter columns packed
into Ahat [D_IN, r_hat] / Bhat [r_hat, D_OUT] and a per-token segment
mask MT [r_hat, T],
    out = x @ W.T + b + ((x @ Ahat) * MT.T) @ Bhat

Sharding: data-parallel over tokens, 1024 tokens per core, zero
collectives.  Since the host knows cu_seqlen values, each core packs
only the adapters of segments overlapping its token range (slots).  Up
to 4 active segments -> r_hat=128; rare draws with more fall back to a
precompiled r_hat=256 variant (always exact).

Precision: the first 2*NP of the 32 k-chunks of the base matmul run as
fp8-e4m3 DoubleRow pair-matmuls (2 MACs/cell/cycle), the rest bf16; the
LoRA down-projection (phase A) runs fully in DoubleRow fp8 (its output
feeds a ~10%-magnitude correction, so fp8 noise there is negligible).
W/Ahat are pre-scaled by 64 into e4m3 range; PSUM accumulates 64*out
and eviction applies *1/64 + bias in one fused DVE op.  Measured
end-to-end rel-err ~1.7e-2 vs the f32 reference (gate 2e-2); bf16-only
is ~2.4e-3.
"""

import numpy as np
import ml_dtypes

# Problem shape (hardcoded per spec nn_BLoraLinear_46471546143180).
T, D_IN, D_OUT, R, M, S = 8192, 4096, 4096, 16, 2, 8
N_CORES = 8
T_C = T // N_CORES
MR = M * R                    # adapter columns per segment (32)

NP = 3                        # k-chunk pairs of the base matmul in fp8 DR
WS = 64.0                     # W / Ahat scale into e4m3 range

BF16 = ml_dtypes.bfloat16
F8 = ml_dtypes.float8_e4m3fn


def _build(t_c, d_in, d_out, r_hat, n_pairs):
    """Per-core Bass/Tile program (same NEFF on all cores).

    DRAM layouts are host-prearranged so every DMA is contiguous per
    partition:
      x8   [128, KX, t_c]       e4m3(x)   x8[p,a,t] = x[tok0+t, a*128+p]
      xb   [128, KB, t_c]       bf16 x, chunks NP2..KX-1 only
      w8   [NB, 128, NP2, 512]  e4m3(64*W.T), chunks 0..NP2-1
      wb   [NB, 128, KB, 512]   bf16(64*W.T), chunks NP2..KX-1
      ah8  [128, KX, r_hat]     e4m3(64*Ahat)
      bh   [128, RC, NB, 512]   bf16 Bhat (unscaled)
      mt   [128, RC, t_c]       bf16 segment mask
      brep [128, d_out]         bf16 bias (unscaled) replicated
      out  [t_c, d_out]         f32
    """
    import concourse.bacc as bacc
    import concourse.mybir as mybir
    from concourse.tile import TileContext

    dt = mybir.dt
    DR = mybir.MatmulPerfMode.DoubleRow
    KX = d_in // 128
    NP2 = 2 * n_pairs
    KB = KX - NP2
    RC = r_hat // 128
    NB = d_out // 512
    MB = t_c // 128
    TB = t_c // 512
    INV = 1.0 / WS

    nc = bacc.Bacc("TRN2", target_bir_lowering=False)

    x8 = nc.dram_tensor("x8", [128, KX, t_c], dt.float8e4, kind="ExternalInput")
    xb = nc.dram_tensor("xb", [128, KB, t_c], dt.bfloat16, kind="ExternalInput")
    w8 = nc.dram_tensor("w8", [NB, 128, NP2, 512], dt.float8e4,
                        kind="ExternalInput")
    wb = nc.dram_tensor("wb", [NB, 128, KB, 512], dt.bfloat16,
                        kind="ExternalInput")
    ah8 = nc.dram_tensor("ah8", [128, KX, r_hat], dt.float8e4,
                         kind="ExternalInput")
    bh = nc.dram_tensor("bh", [128, RC, NB, 512], dt.bfloat16,
                        kind="ExternalInput")
    mt = nc.dram_tensor("mt", [128, RC, t_c], dt.bfloat16, kind="ExternalInput")
    brep = nc.dram_tensor("brep", [128, d_out], dt.bfloat16,
                          kind="ExternalInput")
    out = nc.dram_tensor("out", [t_c, d_out], dt.float32, kind="ExternalOutput")

    with TileContext(nc) as tc:
        with tc.tile_pool(name="resident", bufs=1) as res_pool, \
             tc.tile_pool(name="wpool", bufs=2) as w_pool, \
             tc.tile_pool(name="ps", bufs=8, space="PSUM") as ps_pool, \
             tc.tile_pool(name="opool", bufs=4) as o_pool:
            x8_sb = res_pool.tile([128, KX, t_c], dt.float8e4, name="x8_sb")
            xb_sb = res_pool.tile([128, KB, t_c], dt.bfloat16, name="xb_sb")
            ah8_sb = res_pool.tile([128, KX, r_hat], dt.float8e4, name="ah8_sb")
            bh_sb = res_pool.tile([128, RC, NB, 512], dt.bfloat16, name="bh_sb")
            mt_sb = res_pool.tile([128, RC, t_c], dt.bfloat16, name="mt_sb")
            ut_sb = res_pool.tile([128, RC, t_c], dt.bfloat16, name="ut_sb")
            brep_sb = res_pool.tile([128, d_out], dt.bfloat16, name="brep_sb")

            def wtiles():
                t8 = w_pool.tile([128, NP2, 512], dt.float8e4, name="w8n",
                                 tag="w8n")
                tbf = w_pool.tile([128, KB, 512], dt.bfloat16, name="wbn",
                                  tag="wbn")
                return t8, tbf

            wn_tiles = {}

            def load_wn(n):
                t8, tbf = wtiles()
                nc.sync.dma_start(out=t8[:], in_=w8[n])
                nc.sync.dma_start(out=tbf[:], in_=wb[n])
                wn_tiles[n] = (t8, tbf)

            # PE warm-up: ~13 no-dep matmuls on a scratch tile fill the
            # initial DMA wait and ramp the HAM clock gate to 8/8 before
            # real work arrives.  Results land in a discarded PSUM bank.
            warm_sb = res_pool.tile([128, 640], dt.bfloat16, name="warm_sb")
            nc.vector.memset(warm_sb[:], 0.0)
            ps_w = ps_pool.tile([128, 512], dt.float32, name="ps_w", tag="ps")
            for i in range(13):
                nc.tensor.matmul(ps_w[:], warm_sb[:, 0:128], warm_sb[:, 128:640],
                                 start=(i == 0), stop=(i == 12))

            # Startup is HBM-bandwidth-bound; issue order tracks the PE's
            # consumption order, and transfers are batched to ~1 MiB (small
            # DMAs are descriptor-dominated: 128 KB ~ 180 GB/s vs 1 MB ~ 340).
            step = 4
            h0 = 512
            t8_0, tb_0 = wtiles()
            nc.sync.dma_start(out=ah8_sb[:], in_=ah8[:])
            nc.sync.dma_start(out=x8_sb[:, 0:16, 0:h0], in_=x8[:, 0:16, 0:h0])
            nc.sync.dma_start(out=t8_0[:], in_=w8[0])
            wn_tiles[0] = (t8_0, tb_0)
            nc.sync.dma_start(out=x8_sb[:, 16:KX, 0:h0], in_=x8[:, 16:KX, 0:h0])
            nc.sync.dma_start(out=mt_sb[:], in_=mt[:])
            for g0 in range(0, KB, 8):
                g1 = min(g0 + 8, KB)
                nc.sync.dma_start(out=tb_0[:, g0:g1, :], in_=wb[0, :, g0:g1, :])
                nc.sync.dma_start(out=xb_sb[:, g0:g1, 0:h0],
                                  in_=xb[:, g0:g1, 0:h0])
            nc.sync.dma_start(out=x8_sb[:, 0:16, h0:], in_=x8[:, 0:16, h0:])
            nc.sync.dma_start(out=x8_sb[:, 16:KX, h0:], in_=x8[:, 16:KX, h0:])
            nc.sync.dma_start(out=bh_sb[:], in_=bh[:])
            nc.sync.dma_start(out=brep_sb[:], in_=brep[:])
            for g0 in range(0, KB, 13):
                g1 = min(g0 + 13, KB)
                nc.sync.dma_start(out=xb_sb[:, g0:g1, h0:],
                                  in_=xb[:, g0:g1, h0:])

            # Phase A (one tb block): uT[j,t] = 64*mask[j,t]*sum_k Ahat[k,j]x[t,k]
            def phase_a(tb):
                for rc in range(RC):
                    ps_u = ps_pool.tile([128, 512], dt.float32, name="ps_u",
                                        tag="ps")
                    for p in range(KX // 2):
                        nc.tensor.matmul(
                            ps_u[:],
                            ah8_sb[:, 2 * p:2 * p + 2, rc * 128:(rc + 1) * 128],
                            x8_sb[:, 2 * p:2 * p + 2, tb * 512:(tb + 1) * 512],
                            start=(p == 0), stop=(p == KX // 2 - 1),
                            perf_mode=DR,
                        )
                    nc.vector.tensor_mul(
                        out=ut_sb[:, rc, tb * 512:(tb + 1) * 512],
                        in0=ps_u[:],
                        in1=mt_sb[:, rc, tb * 512:(tb + 1) * 512],
                    )

            def lora_mms(n, m, ps_o):
                for r in range(RC):
                    nc.tensor.matmul(
                        ps_o[:],
                        ut_sb[:, r, m * 128:(m + 1) * 128],
                        bh_sb[:, r, n, :],
                        start=False, stop=(r == RC - 1),
                    )

            def evict(n, m, ps_o):
                o_sb = o_pool.tile([128, 512], dt.float32, name="o_sb")
                nc.vector.scalar_tensor_tensor(
                    out=o_sb[:], in0=ps_o[:], scalar=INV,
                    in1=brep_sb[:, n * 512:(n + 1) * 512],
                    op0=mybir.AluOpType.mult, op1=mybir.AluOpType.add,
                )
                nc.sync.dma_start(
                    out=out[m * 128:(m + 1) * 128, n * 512:(n + 1) * 512],
                    in_=o_sb[:],
                )

            # Phase B tile: psum = 64*(x@W.T + u@Bhat)[m-tile, n-tile]
            def phase_b_tile(n, m, w8n, wbn):
                ps_o = ps_pool.tile([128, 512], dt.float32, name="ps_o",
                                    tag="ps")
                for p in range(n_pairs):
                    nc.tensor.matmul(
                        ps_o[:],
                        x8_sb[:, 2 * p:2 * p + 2, m * 128:(m + 1) * 128],
                        w8n[:, 2 * p:2 * p + 2, :],
                        start=(p == 0), stop=False, perf_mode=DR,
                    )
                for kb in range(KB):
                    nc.tensor.matmul(
                        ps_o[:],
                        xb_sb[:, kb, m * 128:(m + 1) * 128],
                        wbn[:, kb, :],
                        start=(n_pairs == 0 and kb == 0), stop=False,
                    )
                lora_mms(n, m, ps_o)
                evict(n, m, ps_o)

            # Prefix: phase A tb0 and phase-B (n=0, m<half) DR parts ride the
            # ah8/x8-h0 windows; then the m0..3 bf16 k-loop tracks the
            # wbn0/xb chunk stream; phase A tb1 and the lora/evicts follow.
            mb_half = MB // TB
            ps_a = [ps_pool.tile([128, 512], dt.float32, name="ps_u", tag="ps")
                    for _ in range(RC)]
            ps_b = [ps_pool.tile([128, 512], dt.float32, name="ps_o", tag="ps")
                    for _ in range(mb_half)]
            for a0 in range(0, KX, step):
                a1 = a0 + step
                for p in range(a0 // 2, a1 // 2):
                    for rc in range(RC):
                        nc.tensor.matmul(
                            ps_a[rc][:],
                            ah8_sb[:, 2 * p:2 * p + 2, rc * 128:(rc + 1) * 128],
                            x8_sb[:, 2 * p:2 * p + 2, 0:512],
                            start=(p == 0), stop=(p == KX // 2 - 1),
                            perf_mode=DR,
                        )
                for m in range(mb_half):
                    for p in range(a0 // 2, min(a1, NP2) // 2):
                        nc.tensor.matmul(
                            ps_b[m][:],
                            x8_sb[:, 2 * p:2 * p + 2, m * 128:(m + 1) * 128],
                            t8_0[:, 2 * p:2 * p + 2, :],
                            start=(p == 0), stop=False, perf_mode=DR,
                        )
            for rc in range(RC):
                nc.vector.tensor_mul(
                    out=ut_sb[:, rc, 0:512], in0=ps_a[rc][:],
                    in1=mt_sb[:, rc, 0:512])
            for kb in range(KB):
                for m in range(mb_half):
                    nc.tensor.matmul(
                        ps_b[m][:],
                        xb_sb[:, kb, m * 128:(m + 1) * 128],
                        tb_0[:, kb, :],
                        start=(n_pairs == 0 and kb == 0), stop=False,
                    )
            for tb in range(1, TB):
                phase_a(tb)
            for m in range(mb_half):
                lora_mms(0, m, ps_b[m])
                evict(0, m, ps_b[m])

            # Steady state: remaining tiles.
            load_wn(1)
            for m in range(mb_half, MB):
                phase_b_tile(0, m, t8_0, tb_0)
            for n in range(1, NB):
                if n + 1 < NB:
                    load_wn(n + 1)
                w8n, wbn = wn_tiles.pop(n)
                for m in range(MB):
                    phase_b_tile(n, m, w8n, wbn)

    nc.compile()
    nc.finalize()
    return nc


def _core_slots(cu, t_c, n_cores, n_slots):
    """Per-core list of segments overlapping the core's token range,
    padded with -1 to n_slots.  Returns None if any core needs more."""
    out = []
    for c in range(n_cores):
        lo, hi = c * t_c, (c + 1) * t_c
        slots = [s for s in range(S) if cu[s] < hi and cu[s + 1] > lo
                 and cu[s + 1] > cu[s]]
        if len(slots) > n_slots:
            return None
        out.append(slots + [-1] * (n_slots - len(slots)))
    return out


def _prep_in_maps(x, W, b, lora_A, lora_B, cu_seqlen):
    x = np.asarray(x, dtype=np.float32)
    W = np.asarray(W, dtype=np.float32)
    b = np.asarray(b, dtype=np.float32)
    lora_A = np.asarray(lora_A, dtype=np.float32)
    lora_B = np.asarray(lora_B, dtype=np.float32)
    cu = np.asarray(cu_seqlen).astype(np.int64)

    # full Ahat[k, j], Bhat[j, d], j = (s*M + m)*R + r
    Ahat = np.transpose(lora_A, (2, 1, 0, 3)).reshape(D_IN, S * MR)
    Bhat = np.transpose(lora_B, (1, 0, 2, 3)).reshape(S * MR, D_OUT).astype(BF16)

    r_hat = 128
    slots = _core_slots(cu, T_C, N_CORES, r_hat // MR)
    if slots is None:
        r_hat = S * MR                                   # 256 fallback
        slots = [list(range(S)) for _ in range(N_CORES)]

    KX = D_IN // 128
    NP2 = 2 * NP
    KB = KX - NP2
    RC = r_hat // 128
    NB = D_OUT // 512

    WT = np.ascontiguousarray(W.T) * np.float32(WS)      # [D_IN, D_OUT] scaled
    w8_host = np.ascontiguousarray(
        WT[:NP2 * 128].astype(F8).reshape(NP2, 128, NB, 512)
        .transpose(2, 1, 0, 3))
    wb_host = np.ascontiguousarray(
        WT[NP2 * 128:].astype(BF16).reshape(KB, 128, NB, 512)
        .transpose(2, 1, 0, 3))
    brep_host = np.ascontiguousarray(
        np.broadcast_to(b.astype(BF16), (128, D_OUT)))

    xT = x.T                                             # [D_IN, T] view
    in_maps = []
    for c in range(N_CORES):
        sl = slice(c * T_C, (c + 1) * T_C)
        xs = xT[:, sl]
        x8_host = np.ascontiguousarray(
            xs.astype(F8).reshape(KX, 128, T_C).transpose(1, 0, 2))
        xb_host = np.ascontiguousarray(
            xs[NP2 * 128:].astype(BF16).reshape(KB, 128, T_C)
            .transpose(1, 0, 2))

        Ah_c = np.zeros((D_IN, r_hat), dtype=np.float32)
        Bh_c = np.zeros((r_hat, D_OUT), dtype=BF16)
        MT_c = np.zeros((r_hat, T_C), dtype=BF16)
        for a, s in enumerate(slots[c]):
            if s < 0:
                continue
            Ah_c[:, a * MR:(a + 1) * MR] = Ahat[:, s * MR:(s + 1) * MR]
            Bh_c[a * MR:(a + 1) * MR, :] = Bhat[s * MR:(s + 1) * MR, :]
            lo = max(int(cu[s]) - c * T_C, 0)
            hi = min(int(cu[s + 1]) - c * T_C, T_C)
            if hi > lo:
                MT_c[a * MR:(a + 1) * MR, lo:hi] = 1.0

        ah8_host = np.ascontiguousarray(
            (Ah_c * np.float32(WS)).astype(F8).reshape(KX, 128, r_hat)
            .transpose(1, 0, 2))
        bh_host = np.ascontiguousarray(
            Bh_c.reshape(RC, 128, NB, 512).transpose(1, 0, 2, 3))
        mt_host = np.ascontiguousarray(
            MT_c.reshape(RC, 128, T_C).transpose(1, 0, 2))
        in_maps.append({
            "x8": x8_host, "xb": xb_host, "w8": w8_host, "wb": wb_host,
            "ah8": ah8_host, "bh": bh_host, "mt": mt_host, "brep": brep_host,
        })
    return in_maps, r_hat


_NC_CACHE = {}


def _get_nc(r_hat):
    key = (T_C, D_IN, D_OUT, r_hat, NP)
    if key not in _NC_CACHE:
        _NC_CACHE[key] = _build(T_C, D_IN, D_OUT, r_hat, NP)
    return _NC_CACHE[key]


def _ensure_axon_hooks():
    """concourse's trace path imports antenv.axon_hooks, which this image
    lacks.  Provide the tiny get/set registry and wire it to the PJRT
    .so's NTFF entry points when available; degrade to a None hook."""
    import sys
    import types
    if "antenv.axon_hooks" in sys.modules:
        return
    try:
        mod = types.ModuleType("antenv.axon_hooks")
        mod._hook = None
        mod.set_axon_ntff_profile_hook = lambda h: setattr(mod, "_hook", h)
        mod.get_axon_ntff_profile_hook = lambda: mod._hook
        sys.modules["antenv.axon_hooks"] = mod
        import antenv
        antenv.axon_hooks = mod
        try:
            from trn_agent_boot.trn_boot import _ntff_profile_via_ctypes
            mod._hook = _ntff_profile_via_ctypes("/opt/axon/libaxon_pjrt.so")
        except Exception:
            pass
    except Exception:
        pass


def run(inputs, trace=False):
    """Run the SPMD kernel on 8 cores; returns (full_output, results_obj)."""
    _ensure_axon_hooks()
    from concourse.bass_utils import run_bass_kernel_spmd

    in_maps, r_hat = _prep_in_maps(**inputs)
    nc = _get_nc(r_hat)
    res = run_bass_kernel_spmd(
        nc, in_maps, core_ids=list(range(N_CORES)), trace=trace)
    out = np.concatenate([r["out"] for r in res.results], axis=0)
    return out, res


def kernel(x, W, b, lora_A, lora_B, cu_seqlen):
    out, _ = run(dict(x=x, W=W, b=b, lora_A=lora_A, lora_B=lora_B,
                      cu_seqlen=cu_seqlen))
    return out


# revision 12
# speedup vs baseline: 1.0216x; 1.0062x over previous
"""Trainium2 Bass kernel for nn_BLoraLinear (batched multi-adapter LoRA linear).

Math:  out = x @ W.T + b + sum_s sum_m mask_s(t) * (x @ A[m,s]) @ B[m,s]

Reformulation (exact): with per-(module,segment) adapter columns packed
into Ahat [D_IN, r_hat] / Bhat [r_hat, D_OUT] and a per-token segment
mask MT [r_hat, T],
    out = x @ W.T + b + ((x @ Ahat) * MT.T) @ Bhat

Sharding: data-parallel over tokens, 1024 tokens per core, zero
collectives.  Since the host knows cu_seqlen values, each core packs
only the adapters of segments overlapping its token range (slots).  Up
to 4 active segments -> r_hat=128; rare draws with more fall back to a
precompiled r_hat=256 variant (always exact).

Precision: the first 2*NP of the 32 k-chunks of the base matmul run as
fp8-e4m3 DoubleRow pair-matmuls (2 MACs/cell/cycle), the rest bf16; the
LoRA down-projection (phase A) runs fully in DoubleRow fp8 (its output
feeds a ~10%-magnitude correction, so fp8 noise there is negligible).
W/Ahat are pre-scaled by 64 into e4m3 range; PSUM accumulates 64*out
and eviction applies *1/64 + bias in one fused DVE op.  Measured
end-to-end rel-err ~1.7e-2 vs the f32 reference (gate 2e-2); bf16-only
is ~2.4e-3.
"""

import numpy as np
import ml_dtypes

# Problem shape (hardcoded per spec nn_BLoraLinear_46471546143180).
T, D_IN, D_OUT, R, M, S = 8192, 4096, 4096, 16, 2, 8
N_CORES = 8
T_C = T // N_CORES
MR = M * R                    # adapter columns per segment (32)

NP = 3                        # k-chunk pairs of the base matmul in fp8 DR
WS = 64.0                     # W / Ahat scale into e4m3 range

BF16 = ml_dtypes.bfloat16
F8 = ml_dtypes.float8_e4m3fn


def _build(t_c, d_in, d_out, r_hat, n_pairs):
    """Per-core Bass/Tile program (same NEFF on all cores).

    DRAM layouts are host-prearranged so every DMA is contiguous per
    partition:
      x8   [128, KX, t_c]       e4m3(x)   x8[p,a,t] = x[tok0+t, a*128+p]
      xb   [128, KB, t_c]       bf16 x, chunks NP2..KX-1 only
      w8   [NB, 128, NP2, 512]  e4m3(64*W.T), chunks 0..NP2-1
      wb   [NB, 128, KB, 512]   bf16(64*W.T), chunks NP2..KX-1
      ah8  [128, KX, r_hat]     e4m3(64*Ahat)
      bh   [128, RC, NB, 512]   bf16 Bhat (unscaled)
      mt   [128, RC, t_c]       bf16 segment mask
      brep [128, d_out]         bf16 bias (unscaled) replicated
      out  [t_c, d_out]         f32
    """
    import concourse.bacc as bacc
    import concourse.mybir as mybir
    from concourse.tile import TileContext

    dt = mybir.dt
    DR = mybir.MatmulPerfMode.DoubleRow
    KX = d_in // 128
    NP2 = 2 * n_pairs
    KB = KX - NP2
    RC = r_hat // 128
    NB = d_out // 512
    MB = t_c // 128
    TB = t_c // 512
    INV = 1.0 / WS

    nc = bacc.Bacc("TRN2", target_bir_lowering=False)

    x8 = nc.dram_tensor("x8", [128, KX, t_c], dt.float8e4, kind="ExternalInput")
    xb = nc.dram_tensor("xb", [128, KB, t_c], dt.bfloat16, kind="ExternalInput")
    w8 = nc.dram_tensor("w8", [NB, 128, NP2, 512], dt.float8e4,
                        kind="ExternalInput")
    wb = nc.dram_tensor("wb", [NB, 128, KB, 512], dt.bfloat16,
                        kind="ExternalInput")
    ah8 = nc.dram_tensor("ah8", [128, KX, r_hat], dt.float8e4,
                         kind="ExternalInput")
    bh = nc.dram_tensor("bh", [128, RC, NB, 512], dt.bfloat16,
                        kind="ExternalInput")
    mt = nc.dram_tensor("mt", [128, RC, t_c], dt.bfloat16, kind="ExternalInput")
    brep = nc.dram_tensor("brep", [128, d_out], dt.bfloat16,
                          kind="ExternalInput")
    out = nc.dram_tensor("out", [t_c, d_out], dt.float32, kind="ExternalOutput")

    with TileContext(nc) as tc:
        with tc.tile_pool(name="resident", bufs=1) as res_pool, \
             tc.tile_pool(name="wpool", bufs=2) as w_pool, \
             tc.tile_pool(name="ps", bufs=8, space="PSUM") as ps_pool, \
             tc.tile_pool(name="opool", bufs=4) as o_pool:
            x8_sb = res_pool.tile([128, KX, t_c], dt.float8e4, name="x8_sb")
            xb_sb = res_pool.tile([128, KB, t_c], dt.bfloat16, name="xb_sb")
            ah8_sb = res_pool.tile([128, KX, r_hat], dt.float8e4, name="ah8_sb")
            bh_sb = res_pool.tile([128, RC, NB, 512], dt.bfloat16, name="bh_sb")
            mt_sb = res_pool.tile([128, RC, t_c], dt.bfloat16, name="mt_sb")
            ut_sb = res_pool.tile([128, RC, t_c], dt.bfloat16, name="ut_sb")
            brep_sb = res_pool.tile([128, d_out], dt.bfloat16, name="brep_sb")

            def wtiles():
                t8 = w_pool.tile([128, NP2, 512], dt.float8e4, name="w8n",
                                 tag="w8n")
                tbf = w_pool.tile([128, KB, 512], dt.bfloat16, name="wbn",
                                  tag="wbn")
                return t8, tbf

            wn_tiles = {}

            def load_wn(n):
                t8, tbf = wtiles()
                nc.sync.dma_start(out=t8[:], in_=w8[n])
                nc.sync.dma_start(out=tbf[:], in_=wb[n])
                wn_tiles[n] = (t8, tbf)

            # PE warm-up: ~13 no-dep matmuls on a scratch tile fill the
            # initial DMA wait and ramp the HAM clock gate to 8/8 before
            # real work arrives.  Results land in a discarded PSUM bank.
            warm_sb = res_pool.tile([128, 640], dt.bfloat16, name="warm_sb")
            nc.vector.memset(warm_sb[:], 0.0)
            ps_w = ps_pool.tile([128, 512], dt.float32, name="ps_w", tag="ps")
            for i in range(13):
                nc.tensor.matmul(ps_w[:], warm_sb[:, 0:128], warm_sb[:, 128:640],
                                 start=(i == 0), stop=(i == 12))

            # Startup is HBM-bandwidth-bound; issue order tracks the PE's
            # consumption order, and transfers are batched to ~1 MiB (small
            # DMAs are descriptor-dominated: 128 KB ~ 180 GB/s vs 1 MB ~ 340).
            step = 4
            h0 = 512
            t8_0, tb_0 = wtiles()
            nc.sync.dma_start(out=ah8_sb[:], in_=ah8[:])
            nc.sync.dma_start(out=x8_sb[:, 0:16, 0:h0], in_=x8[:, 0:16, 0:h0])
            nc.sync.dma_start(out=t8_0[:], in_=w8[0])
            wn_tiles[0] = (t8_0, tb_0)
            nc.sync.dma_start(out=x8_sb[:, 16:KX, 0:h0], in_=x8[:, 16:KX, 0:h0])
            nc.sync.dma_start(out=mt_sb[:], in_=mt[:])
            for g0, g1 in [(0, 4), (4, 8), (8, 16), (16, 24), (24, KB)]:
                nc.sync.dma_start(out=tb_0[:, g0:g1, :], in_=wb[0, :, g0:g1, :])
                nc.sync.dma_start(out=xb_sb[:, g0:g1, 0:h0],
                                  in_=xb[:, g0:g1, 0:h0])
            nc.sync.dma_start(out=x8_sb[:, 0:16, h0:], in_=x8[:, 0:16, h0:])
            nc.sync.dma_start(out=x8_sb[:, 16:KX, h0:], in_=x8[:, 16:KX, h0:])
            nc.sync.dma_start(out=bh_sb[:], in_=bh[:])
            nc.sync.dma_start(out=brep_sb[:], in_=brep[:])
            for g0 in range(0, KB, 13):
                g1 = min(g0 + 13, KB)
                nc.sync.dma_start(out=xb_sb[:, g0:g1, h0:],
                                  in_=xb[:, g0:g1, h0:])

            # Phase A (one tb block): uT[j,t] = 64*mask[j,t]*sum_k Ahat[k,j]x[t,k]
            def phase_a(tb):
                for rc in range(RC):
                    ps_u = ps_pool.tile([128, 512], dt.float32, name="ps_u",
                                        tag="ps")
                    for p in range(KX // 2):
                        nc.tensor.matmul(
                            ps_u[:],
                            ah8_sb[:, 2 * p:2 * p + 2, rc * 128:(rc + 1) * 128],
                            x8_sb[:, 2 * p:2 * p + 2, tb * 512:(tb + 1) * 512],
                            start=(p == 0), stop=(p == KX // 2 - 1),
                            perf_mode=DR,
                        )
                    nc.vector.tensor_mul(
                        out=ut_sb[:, rc, tb * 512:(tb + 1) * 512],
                        in0=ps_u[:],
                        in1=mt_sb[:, rc, tb * 512:(tb + 1) * 512],
                    )

            def lora_mms(n, m, ps_o):
                for r in range(RC):
                    nc.tensor.matmul(
                        ps_o[:],
                        ut_sb[:, r, m * 128:(m + 1) * 128],
                        bh_sb[:, r, n, :],
                        start=False, stop=(r == RC - 1),
                    )

            def evict(n, m, ps_o):
                o_sb = o_pool.tile([128, 512], dt.float32, name="o_sb")
                nc.vector.scalar_tensor_tensor(
                    out=o_sb[:], in0=ps_o[:], scalar=INV,
                    in1=brep_sb[:, n * 512:(n + 1) * 512],
                    op0=mybir.AluOpType.mult, op1=mybir.AluOpType.add,
                )
                nc.sync.dma_start(
                    out=out[m * 128:(m + 1) * 128, n * 512:(n + 1) * 512],
                    in_=o_sb[:],
                )

            # Phase B tile: psum = 64*(x@W.T + u@Bhat)[m-tile, n-tile]
            def phase_b_tile(n, m, w8n, wbn):
                ps_o = ps_pool.tile([128, 512], dt.float32, name="ps_o",
                                    tag="ps")
                for p in range(n_pairs):
                    nc.tensor.matmul(
                        ps_o[:],
                        x8_sb[:, 2 * p:2 * p + 2, m * 128:(m + 1) * 128],
                        w8n[:, 2 * p:2 * p + 2, :],
                        start=(p == 0), stop=False, perf_mode=DR,
                    )
                for kb in range(KB):
                    nc.tensor.matmul(
                        ps_o[:],
                        xb_sb[:, kb, m * 128:(m + 1) * 128],
                        wbn[:, kb, :],
                        start=(n_pairs == 0 and kb == 0), stop=False,
                    )
                lora_mms(n, m, ps_o)
                evict(n, m, ps_o)

            # Prefix: phase A tb0 and phase-B (n=0, m<half) DR parts ride the
            # ah8/x8-h0 windows; then the m0..3 bf16 k-loop tracks the
            # wbn0/xb chunk stream; phase A tb1 and the lora/evicts follow.
            mb_half = MB // TB
            ps_a = [ps_pool.tile([128, 512], dt.float32, name="ps_u", tag="ps")
                    for _ in range(RC)]
            ps_b = [ps_pool.tile([128, 512], dt.float32, name="ps_o", tag="ps")
                    for _ in range(mb_half)]
            for a0 in range(0, KX, step):
                a1 = a0 + step
                for p in range(a0 // 2, a1 // 2):
                    for rc in range(RC):
                        nc.tensor.matmul(
                            ps_a[rc][:],
                            ah8_sb[:, 2 * p:2 * p + 2, rc * 128:(rc + 1) * 128],
                            x8_sb[:, 2 * p:2 * p + 2, 0:512],
                            start=(p == 0), stop=(p == KX // 2 - 1),
                            perf_mode=DR,
                        )
                for m in range(mb_half):
                    for p in range(a0 // 2, min(a1, NP2) // 2):
                        nc.tensor.matmul(
                            ps_b[m][:],
                            x8_sb[:, 2 * p:2 * p + 2, m * 128:(m + 1) * 128],
                            t8_0[:, 2 * p:2 * p + 2, :],
                            start=(p == 0), stop=False, perf_mode=DR,
                        )
            for rc in range(RC):
                nc.vector.tensor_mul(
                    out=ut_sb[:, rc, 0:512], in0=ps_a[rc][:],
                    in1=mt_sb[:, rc, 0:512])
            for kb in range(KB):
                for m in range(mb_half):
                    nc.tensor.matmul(
                        ps_b[m][:],
                        xb_sb[:, kb, m * 128:(m + 1) * 128],
                        tb_0[:, kb, :],
                        start=(n_pairs == 0 and kb == 0), stop=False,
                    )
            for tb in range(1, TB):
                phase_a(tb)
            for m in range(mb_half):
                lora_mms(0, m, ps_b[m])
                evict(0, m, ps_b[m])

            # Steady state: remaining tiles.
            load_wn(1)
            for m in range(mb_half, MB):
                phase_b_tile(0, m, t8_0, tb_0)
            for n in range(1, NB):
                if n + 1 < NB:
                    load_wn(n + 1)
                w8n, wbn = wn_tiles.pop(n)
                for m in range(MB):
                    phase_b_tile(n, m, w8n, wbn)

    nc.compile()
    nc.finalize()
    return nc


def _core_slots(cu, t_c, n_cores, n_slots):
    """Per-core list of segments overlapping the core's token range,
    padded with -1 to n_slots.  Returns None if any core needs more."""
    out = []
    for c in range(n_cores):
        lo, hi = c * t_c, (c + 1) * t_c
        slots = [s for s in range(S) if cu[s] < hi and cu[s + 1] > lo
                 and cu[s + 1] > cu[s]]
        if len(slots) > n_slots:
            return None
        out.append(slots + [-1] * (n_slots - len(slots)))
    return out


def _prep_in_maps(x, W, b, lora_A, lora_B, cu_seqlen):
    x = np.asarray(x, dtype=np.float32)
    W = np.asarray(W, dtype=np.float32)
    b = np.asarray(b, dtype=np.float32)
    lora_A = np.asarray(lora_A, dtype=np.float32)
    lora_B = np.asarray(lora_B, dtype=np.float32)
    cu = np.asarray(cu_seqlen).astype(np.int64)

    # full Ahat[k, j], Bhat[j, d], j = (s*M + m)*R + r
    Ahat = np.transpose(lora_A, (2, 1, 0, 3)).reshape(D_IN, S * MR)
    Bhat = np.transpose(lora_B, (1, 0, 2, 3)).reshape(S * MR, D_OUT).astype(BF16)

    r_hat = 128
    slots = _core_slots(cu, T_C, N_CORES, r_hat // MR)
    if slots is None:
        r_hat = S * MR                                   # 256 fallback
        slots = [list(range(S)) for _ in range(N_CORES)]

    KX = D_IN // 128
    NP2 = 2 * NP
    KB = KX - NP2
    RC = r_hat // 128
    NB = D_OUT // 512

    WT = np.ascontiguousarray(W.T) * np.float32(WS)      # [D_IN, D_OUT] scaled
    w8_host = np.ascontiguousarray(
        WT[:NP2 * 128].astype(F8).reshape(NP2, 128, NB, 512)
        .transpose(2, 1, 0, 3))
    wb_host = np.ascontiguousarray(
        WT[NP2 * 128:].astype(BF16).reshape(KB, 128, NB, 512)
        .transpose(2, 1, 0, 3))
    brep_host = np.ascontiguousarray(
        np.broadcast_to(b.astype(BF16), (128, D_OUT)))

    xT = x.T                                             # [D_IN, T] view
    in_maps = []
    for c in range(N_CORES):
        sl = slice(c * T_C, (c + 1) * T_C)
        xs = xT[:, sl]
        x8_host = np.ascontiguousarray(
            xs.astype(F8).reshape(KX, 128, T_C).transpose(1, 0, 2))
        xb_host = np.ascontiguousarray(
            xs[NP2 * 128:].astype(BF16).reshape(KB, 128, T_C)
            .transpose(1, 0, 2))

        Ah_c = np.zeros((D_IN, r_hat), dtype=np.float32)
        Bh_c = np.zeros((r_hat, D_OUT), dtype=BF16)
        MT_c = np.zeros((r_hat, T_C), dtype=BF16)
        for a, s in enumerate(slots[c]):
            if s < 0:
                continue
            Ah_c[:, a * MR:(a + 1) * MR] = Ahat[:, s * MR:(s + 1) * MR]
            Bh_c[a * MR:(a + 1) * MR, :] = Bhat[s * MR:(s + 1) * MR, :]
            lo = max(int(cu[s]) - c * T_C, 0)
            hi = min(int(cu[s + 1]) - c * T_C, T_C)
            if hi > lo:
                MT_c[a * MR:(a + 1) * MR, lo:hi] = 1.0

        ah8_host = np.ascontiguousarray(
            (Ah_c * np.float32(WS)).astype(F8).reshape(KX, 128, r_hat)
            .transpose(1, 0, 2))
        bh_host = np.ascontiguousarray(
            Bh_c.reshape(RC, 128, NB, 512).transpose(1, 0, 2, 3))
        mt_host = np.ascontiguousarray(
            MT_c.reshape(RC, 128, T_C).transpose(1, 0, 2))
        in_maps.append({
            "x8": x8_host, "xb": xb_host, "w8": w8_host, "wb": wb_host,
            "ah8": ah8_host, "bh": bh_host, "mt": mt_host, "brep": brep_host,
        })
    return in_maps, r_hat


_NC_CACHE = {}


def _get_nc(r_hat):
    key = (T_C, D_IN, D_OUT, r_hat, NP)
    if key not in _NC_CACHE:
        _NC_CACHE[key] = _build(T_C, D_IN, D_OUT, r_hat, NP)
    return _NC_CACHE[key]


def _ensure_axon_hooks():
    """concourse's trace path imports antenv.axon_hooks, which this image
    lacks.  Provide the tiny get/set registry and wire it to the PJRT
    .so's NTFF entry points when available; degrade to a None hook."""
    import sys
    import types
    if "antenv.axon_hooks" in sys.modules:
        return
    try:
        mod = types.ModuleType("antenv.axon_hooks")
        mod._hook = None
        mod.set_axon_ntff_profile_hook = lambda h: setattr(mod, "_hook", h)
        mod.get_axon_ntff_profile_hook = lambda: mod._hook
        sys.modules["antenv.axon_hooks"] = mod
        import antenv
        antenv.axon_hooks = mod
        try:
            from trn_agent_boot.trn_boot import _ntff_profile_via_ctypes
            mod._hook = _ntff_profile_via_ctypes("/opt/axon/libaxon_pjrt.so")
        except Exception:
            pass
    except Exception:
        pass


def run(inputs, trace=False):
    """Run the SPMD kernel on 8 cores; returns (full_output, results_obj)."""
    _ensure_axon_hooks()
    from concourse.bass_utils import run_bass_kernel_spmd

    in_maps, r_hat = _prep_in_maps(**inputs)
    nc = _get_nc(r_hat)
    res = run_bass_kernel_spmd(
        nc, in_maps, core_ids=list(range(N_CORES)), trace=trace)
    out = np.concatenate([r["out"] for r in res.results], axis=0)
    return out, res


def kernel(x, W, b, lora_A, lora_B, cu_seqlen):
    out, _ = run(dict(x=x, W=W, b=b, lora_A=lora_A, lora_B=lora_B,
                      cu_seqlen=cu_seqlen))
    return out
